# revision 23
# baseline (speedup 1.0000x reference)
"""HardAndLayer on 8 Trainium2 NeuronCores.

out[l] = AND_d (x[d] OR NOT w[l,d])  ==  no d with (w[l,d] AND NOT x[d])

Strategy (per sharding hint): shard bit_weights row-wise (neuron dim) across
8 cores, x replicated, no collectives.

Wire format: the bool tensors are bit-packed on the host, 31 bools per
32-bit word with bit 30 (top fp32 exponent bit) forced to zero, so no word
can form a NaN/Inf pattern. Each core moves ~1.2 MB instead of 8 MB over
HBM. On device a custom fused DVE op computes, per neuron row,
    acc[p] = fold_logical_or_j (w_packed[p, j] BITWISE_AND notx_packed[j])
in a single pass: the streams are declared fp32 (identity converter — no
int conversion), BITWISE_AND preserves raw bits, and LOGICAL_OR folds on
bit-pattern truthiness (HW-verified: -0.0-only words count as violations).
out[l] = (acc == 0), applied on the host to the DMA'd per-neuron flags.
All reduction math happens on device; host packing/relabeling is layout
only.

Layout: partition p of a core holds its 8 consecutive neuron rows
(8 KB contiguous per partition) so the weight shard arrives in a few large
DMAs, and res[p, b] = out[8p + b] is identity-ordered on the host.
"""

import numpy as np

L = 8192
D = 8192
NCORES = 8
LSH = L // NCORES  # 1024 neuron rows per core
PAYLOAD = 31  # bits per packed word (bit 30 held zero -> never NaN/Inf)
WPK = -(-D // PAYLOAD)  # 265 packed words per neuron row
DPAD = WPK * PAYLOAD
# payload bit positions: 0..29 and 31 (skip bit 30)
_BITPOS = list(range(30)) + [31]
NB = LSH // 128  # 8 neuron rows per partition
# Per-partition DRAM layout: [notx | row0 | ... | row7], 9*WPK words
# contiguous per partition. Chunks in row-units (chunk 0 carries notx).
CHUNK_UNITS = (3, 2, 2, 1, 1)
CHUNK_COLS = tuple(u * WPK for u in CHUNK_UNITS)

_compiled = None
_custom_op = None


def _register_custom_op():
    """Register the fused AND+any op in the custom-DVE table (idempotent)."""
    global _custom_op
    if _custom_op is not None:
        return _custom_op
    from concourse import dve_ops
    from concourse.dve_spec import Spec, Src0, Src1, Zero, Bin, lower
    from concourse.dve_uop import AluOp, DveOpSpec

    name = "AND_ANY_ANT"
    for o in dve_ops.OPS:
        if o.name == name:
            _custom_op = o
            return o

    def _ref(in0, in1, c0, c1, c2):
        a = in0.view(np.uint32) & in1.view(np.uint32)
        acc = (
            (a.reshape(a.shape[0], -1) != 0)
            .any(axis=-1, keepdims=True)
            .astype(np.float32)
        )
        return a.view(np.float32), acc

    spec = Spec(
        body=Bin(AluOp.BITWISE_AND, Src0, Src1),
        accum=AluOp.LOGICAL_OR,
        accum_init=Zero,
        reference=_ref,
    )
    shas = {}
    for ver in ("v3", "v4"):
        try:
            uops = lower(spec, ver=ver)
            shas[ver] = DveOpSpec(name=name, uops=uops, rd1_en=True).sha(ver)
        except Exception:
            pass
    op = dve_ops.DveOp(name, spec, subdim=False, uops_sha=shas)
    dve_ops.OPS.append(op)
    dve_ops._SUB_OPCODE_FOR_NAME[name] = (
        dve_ops._CUSTOM_DVE_ROW_BASE + len(dve_ops.OPS) - 1
    )
    dve_ops.CUSTOM_DVE_SPECS[name] = spec
    _custom_op = op
    return op


def _build():
    import concourse.bacc as bacc
    import concourse.mybir as mybir
    from concourse import tile
    from concourse.instruction_name_ordered_set import InstructionNameOrderedSet

    op = _register_custom_op()

    nc = bacc.Bacc(
        "TRN2",
        target_bir_lowering=False,
        debug=False,
        enable_asserts=False,
        num_devices=NCORES,
    )
    TOT = (NB + 1) * WPK
    wx = nc.dram_tensor("wx", [128, TOT], mybir.dt.float32, kind="ExternalInput")
    # Output shaped for kv_writeback: [batch=1, d_head_inner=128,
    # d_head_outer=1, n_ctx=NB]; res[0, p, 0, b] = flag of neuron 8p + b.
    res = nc.dram_tensor("res", [1, 128, 1, NB], mybir.dt.float32, kind="ExternalOutput")

    with tile.TileContext(nc) as tc:
        # One pool => one sem-init memset in the Pool preamble; every extra
        # pool delays the entry barrier (and thus the first DMA) by ~60-100ns.
        with tc.tile_pool(name="pool", bufs=1) as wpool:
            mpool = small = wpool
            # acc as [128, dho=1, batch=1, ncn=NB] so its AP is directly the
            # kv_writeback source layout.
            acc = small.tile([128, 1, 1, NB], mybir.dt.float32)
            ctx_idxs = small.tile([128, 1], mybir.dt.int32)
            # ctx_idxs is metadata consumed at prep (desc-gen) time.
            ctx_memset = nc.gpsimd.memset(ctx_idxs[:], 0)
            dma_sem = nc.alloc_semaphore("out_dma")
            tiles = []
            dma_insts = []
            c0 = 0
            for ci, cw in enumerate(CHUNK_COLS):
                wt = wpool.tile([128, cw], mybir.dt.float32, tag=f"wt{ci}")
                dma_eng = nc.sync if ci % 2 == 0 else nc.scalar
                dma_insts.append(dma_eng.dma_start(wt[:], wx[:, c0 : c0 + cw]).ins)
                tiles.append((wt, c0, cw))
                c0 += cw
            nx_ap = tiles[0][0][:, 0:WPK]  # notx lives in chunk 0, col 0
            for gb in range(NB):
                col = (gb + 1) * WPK  # global word offset of neuron row gb
                for wt, tc0, tcw in tiles:
                    if tc0 <= col < tc0 + tcw:
                        in0 = wt[:, col - tc0 : col - tc0 + WPK]
                        break
                m = mpool.tile([128, WPK], mybir.dt.float32, tag=f"m{gb % 2}")
                nc.vector._custom_dve(
                    op,
                    out=m[:],
                    in0=in0,
                    in1=nx_ap,
                    accum_out=acc[:, 0, 0, gb : gb + 1],
                )
            # The prep only generates descriptors (reads ctx_idxs + AP
            # metadata, not acc data — the DMA reads acc when the trigger
            # fires). Tile does not defer kv_writeback's source deps the way
            # it does for scatter_add, so move the acc RAW deps from the prep
            # to the trigger by hand: the ~1us desc-gen then runs on the idle
            # Pool engine at kernel start, and the trigger waits for the last
            # accumulator write. The race detector treats gen_mode==1 source
            # reads as deferred to the trigger already.
            prep = nc.gpsimd.kv_writeback(
                res[:, :, :, :],
                acc[:, :, :, :],
                ctx_idxs[:],
                prepare_only=True,
                sem=dma_sem,
            )
            trig = nc.gpsimd.trigger_dma(count=None)
            ctx_writer = {ctx_memset.ins.name}
            deferred = InstructionNameOrderedSet()
            kept = InstructionNameOrderedSet()
            for n in prep.ins.sync_dependency_names():
                (kept if n in ctx_writer else deferred).add(n)
            prep.ins.set_sync_dependencies(kept)
            trig.ins.add_sync_dependencies_from(deferred)

    # Hoist chunk 0's DMACopy (no waits; Tile-assigned completion sem intact)
    # into the entry block ahead of SP's barrier drain: descriptor gen and the
    # 650ns DGE launch then overlap the entry barrier, starting the HBM
    # stream ~640ns earlier. Execution semantics are unchanged — consumers
    # still wait on the DMA completion semaphore, which fires microseconds
    # after anything the barrier orders.
    fn = nc.m.functions[0]
    entry = fn.blocks[0]
    hoist = dma_insts[0]
    for b in fn.blocks:
        names = [i.name for i in b.instructions]
        if hoist.name in names:
            assert not (hoist.sync_info and hoist.sync_info.on_wait), (
                "chunk0 DMA grew a wait; unsafe to hoist"
            )
            b.instructions.remove(hoist)
            entry_names = [
                (j, i)
                for j, i in enumerate(entry.instructions)
                if isinstance(i, mybir.InstDrain)
                and i.engine == mybir.EngineType.SP
            ]
            assert entry_names, "SP barrier drain not found in entry block"
            entry.instructions.insert(entry_names[0][0], hoist)
            break

    # Tile's exit protocol waits on its DMASW0 lane semaphore for the SWDGE
    # writeback, but the descriptor carries a single completion sem —
    # on_update[0], which the sem= kwarg filled with our placeholder. Point
    # on_update[0] at the DMASW0 lane sem instead so the DMA completion
    # (data landed + sem propagation) is what releases the exit barrier, in
    # both the executor and the timeline cost model.
    fn = nc.m.functions[0]
    dmasw = None
    for b in fn.blocks:
        for inst in b.instructions:
            si = inst.sync_info
            if not si:
                continue
            for w in si.on_wait or []:
                nm = getattr(w, "ant_name", "") or ""
                if nm.startswith("DMASW"):
                    dmasw = w
    assert dmasw is not None, "no DMASW exit wait found"
    for b in fn.blocks:
        for inst in b.instructions:
            if isinstance(inst, mybir.InstKVWritebackAnt):
                inst.sync_info.on_update[0] = mybir.SyncUpdate(
                    sync_type="semaphore",
                    id=dmasw.id,
                    ant_name=dmasw.ant_name,
                    update_mode="sem-add-imm",
                    update_value=16,
                )

    nc.compile()
    return nc


def _pack31(bits):
    """bits [..., D] uint8 -> [..., WPK] float32-viewed words, 31 bits/word
    at positions 0..29 and 31 (bit 30 always zero -> never NaN/Inf)."""
    lead = bits.shape[:-1]
    b32 = np.zeros(lead + (WPK, 32), dtype=np.uint8)
    pad = np.zeros(lead + (DPAD,), dtype=np.uint8)
    pad[..., :D] = bits
    pad = pad.reshape(lead + (WPK, PAYLOAD))
    b32[..., :30] = pad[..., :30]
    b32[..., 31] = pad[..., 30]
    words = np.packbits(b32.reshape(lead + (WPK * 32,)), axis=-1, bitorder="little")
    return words.view(np.uint32).view(np.float32)


def _pack_inputs(x, bit_weights):
    x = np.asarray(x).astype(np.uint8)
    bw = np.ascontiguousarray(np.asarray(bit_weights).astype(np.uint8))
    notx = (1 - x).astype(np.uint8)
    nxp = _pack31(notx)  # [WPK]
    wp = _pack31(bw)  # [L, WPK]
    in_maps = []
    for i in range(NCORES):
        shard = wp[i * LSH : (i + 1) * LSH].reshape(128, NB, WPK)
        wx = np.empty((128, NB + 1, WPK), dtype=np.float32)
        wx[:, 0, :] = nxp
        wx[:, 1:, :] = shard
        in_maps.append({"wx": wx.reshape(128, (NB + 1) * WPK)})
    return in_maps


def _gather(results):
    outs = []
    for i in range(NCORES):
        # [1, 128, 1, NB] fp32 violation flags; res[0, p, 0, b] covers neuron
        # 8p + b, flag == 0.0 means no violated requirement -> output True
        res = results[i]["res"].reshape(128, NB)
        outs.append(res.reshape(-1) == 0.0)
    return np.concatenate(outs).astype(np.bool_)


def _get_compiled():
    global _compiled
    if _compiled is None:
        _compiled = _build()
    return _compiled


def kernel(x, bit_weights):
    from concourse import bass_utils

    nc = _get_compiled()
    in_maps = _pack_inputs(x, bit_weights)
    r = bass_utils.run_bass_kernel_spmd(nc, in_maps, core_ids=list(range(NCORES)))
    return _gather(r.results)



# revision 24
# speedup vs baseline: 1.0852x; 1.0852x over previous
"""HardAndLayer on 8 Trainium2 NeuronCores.

out[l] = AND_d (x[d] OR NOT w[l,d])  ==  no d with (w[l,d] AND NOT x[d])

Strategy (per sharding hint): shard bit_weights row-wise (neuron dim) across
8 cores, x replicated, no collectives.

Wire format: the bool tensors are bit-packed on the host, 31 bools per
32-bit word with bit 30 (top fp32 exponent bit) forced to zero, so no word
can form a NaN/Inf pattern. Each core moves ~1.2 MB instead of 8 MB over
HBM. On device a custom fused DVE op computes, per neuron row,
    acc[p] = fold_logical_or_j (w_packed[p, j] BITWISE_AND notx_packed[j])
in a single pass: the streams are declared fp32 (identity converter — no
int conversion), BITWISE_AND preserves raw bits, and LOGICAL_OR folds on
bit-pattern truthiness (HW-verified: -0.0-only words count as violations).
out[l] = (acc == 0), applied on the host to the DMA'd per-neuron flags.
All reduction math happens on device; host packing/relabeling is layout
only.

Layout: partition p of a core holds its 8 consecutive neuron rows
(8 KB contiguous per partition) so the weight shard arrives in a few large
DMAs, and res[p, b] = out[8p + b] is identity-ordered on the host.
"""

import numpy as np

L = 8192
D = 8192
NCORES = 8
LSH = L // NCORES  # 1024 neuron rows per core
PAYLOAD = 31  # bits per packed word (bit 30 held zero -> never NaN/Inf)
WPK = -(-D // PAYLOAD)  # 265 packed words per neuron row
DPAD = WPK * PAYLOAD
# payload bit positions: 0..29 and 31 (skip bit 30)
_BITPOS = list(range(30)) + [31]
NB = LSH // 128  # 8 neuron rows per partition
# Per-partition DRAM layout: [notx | row0 | ... | row7], 9*WPK words
# contiguous per partition. Chunks in row-units (chunk 0 carries notx).
CHUNK_UNITS = (3, 2, 2, 1, 1)
CHUNK_COLS = tuple(u * WPK for u in CHUNK_UNITS)

_compiled = None
_custom_op = None


def _register_custom_op():
    """Register the fused AND+any op in the custom-DVE table (idempotent)."""
    global _custom_op
    if _custom_op is not None:
        return _custom_op
    from concourse import dve_ops
    from concourse.dve_spec import Spec, Src0, Src1, Zero, Bin, lower
    from concourse.dve_uop import AluOp, DveOpSpec

    name = "AND_ANY_ANT"
    for o in dve_ops.OPS:
        if o.name == name:
            _custom_op = o
            return o

    def _ref(in0, in1, c0, c1, c2):
        a = in0.view(np.uint32) & in1.view(np.uint32)
        acc = (
            (a.reshape(a.shape[0], -1) != 0)
            .any(axis=-1, keepdims=True)
            .astype(np.float32)
        )
        return a.view(np.float32), acc

    spec = Spec(
        body=Bin(AluOp.BITWISE_AND, Src0, Src1),
        accum=AluOp.LOGICAL_OR,
        accum_init=Zero,
        reference=_ref,
    )
    shas = {}
    for ver in ("v3", "v4"):
        try:
            uops = lower(spec, ver=ver)
            shas[ver] = DveOpSpec(name=name, uops=uops, rd1_en=True).sha(ver)
        except Exception:
            pass
    op = dve_ops.DveOp(name, spec, subdim=False, uops_sha=shas)
    dve_ops.OPS.append(op)
    dve_ops._SUB_OPCODE_FOR_NAME[name] = (
        dve_ops._CUSTOM_DVE_ROW_BASE + len(dve_ops.OPS) - 1
    )
    dve_ops.CUSTOM_DVE_SPECS[name] = spec
    _custom_op = op
    return op


def _build():
    import concourse.bacc as bacc
    import concourse.mybir as mybir
    from concourse import tile
    from concourse.instruction_name_ordered_set import InstructionNameOrderedSet

    op = _register_custom_op()

    nc = bacc.Bacc(
        "TRN2",
        target_bir_lowering=False,
        debug=False,
        enable_asserts=False,
        num_devices=NCORES,
    )
    TOT = (NB + 1) * WPK
    wx = nc.dram_tensor("wx", [128, TOT], mybir.dt.float32, kind="ExternalInput")
    # Output shaped for kv_writeback: [batch=1, d_head_inner=128,
    # d_head_outer=1, n_ctx=NB]; res[0, p, 0, b] = flag of neuron 8p + b.
    res = nc.dram_tensor("res", [1, 128, 1, NB], mybir.dt.float32, kind="ExternalOutput")

    with tile.TileContext(nc) as tc:
        # One pool => one sem-init memset in the Pool preamble; every extra
        # pool delays the entry barrier (and thus the first DMA) by ~60-100ns.
        with tc.tile_pool(name="pool", bufs=1) as wpool:
            mpool = small = wpool
            # acc as [128, dho=1, batch=1, ncn=NB] so its AP is directly the
            # kv_writeback source layout.
            acc = small.tile([128, 1, 1, NB], mybir.dt.float32)
            ctx_idxs = small.tile([128, 1], mybir.dt.int32)
            # ctx_idxs is metadata consumed at prep (desc-gen) time.
            ctx_memset = nc.gpsimd.memset(ctx_idxs[:], 0)
            dma_sem = nc.alloc_semaphore("out_dma")
            tiles = []
            dma_insts = []
            c0 = 0
            for ci, cw in enumerate(CHUNK_COLS):
                wt = wpool.tile([128, cw], mybir.dt.float32, tag=f"wt{ci}")
                # chunk 0 is hoisted to the entry block below, so after the
                # barrier SP's first tile DMA is chunk 1 — SP reaches the
                # HWDGE a hair before Act, so give SP chunks 1,3 and Act 2,4
                # to keep HBM arrival order matching DVE consumption order.
                dma_eng = nc.sync if ci in (0, 1, 3) else nc.scalar
                dma_insts.append(dma_eng.dma_start(wt[:], wx[:, c0 : c0 + cw]).ins)
                tiles.append((wt, c0, cw))
                c0 += cw
            nx_ap = tiles[0][0][:, 0:WPK]  # notx lives in chunk 0, col 0
            for gb in range(NB):
                col = (gb + 1) * WPK  # global word offset of neuron row gb
                for wt, tc0, tcw in tiles:
                    if tc0 <= col < tc0 + tcw:
                        in0 = wt[:, col - tc0 : col - tc0 + WPK]
                        break
                m = mpool.tile([128, WPK], mybir.dt.float32, tag=f"m{gb % 2}")
                nc.vector._custom_dve(
                    op,
                    out=m[:],
                    in0=in0,
                    in1=nx_ap,
                    accum_out=acc[:, 0, 0, gb : gb + 1],
                )
            # The prep only generates descriptors (reads ctx_idxs + AP
            # metadata, not acc data — the DMA reads acc when the trigger
            # fires). Tile does not defer kv_writeback's source deps the way
            # it does for scatter_add, so move the acc RAW deps from the prep
            # to the trigger by hand: the ~1us desc-gen then runs on the idle
            # Pool engine at kernel start, and the trigger waits for the last
            # accumulator write. The race detector treats gen_mode==1 source
            # reads as deferred to the trigger already.
            prep = nc.gpsimd.kv_writeback(
                res[:, :, :, :],
                acc[:, :, :, :],
                ctx_idxs[:],
                prepare_only=True,
                sem=dma_sem,
            )
            trig = nc.gpsimd.trigger_dma(count=None)
            ctx_writer = {ctx_memset.ins.name}
            deferred = InstructionNameOrderedSet()
            kept = InstructionNameOrderedSet()
            for n in prep.ins.sync_dependency_names():
                (kept if n in ctx_writer else deferred).add(n)
            prep.ins.set_sync_dependencies(kept)
            trig.ins.add_sync_dependencies_from(deferred)

    # Hoist chunk 0's DMACopy (no waits; Tile-assigned completion sem intact)
    # into the entry block ahead of SP's barrier drain: descriptor gen and the
    # 650ns DGE launch then overlap the entry barrier, starting the HBM
    # stream ~640ns earlier. Execution semantics are unchanged — consumers
    # still wait on the DMA completion semaphore, which fires microseconds
    # after anything the barrier orders.
    fn = nc.m.functions[0]
    entry = fn.blocks[0]
    hoist = dma_insts[0]
    for b in fn.blocks:
        names = [i.name for i in b.instructions]
        if hoist.name in names:
            assert not (hoist.sync_info and hoist.sync_info.on_wait), (
                "chunk0 DMA grew a wait; unsafe to hoist"
            )
            b.instructions.remove(hoist)
            entry_names = [
                (j, i)
                for j, i in enumerate(entry.instructions)
                if isinstance(i, mybir.InstDrain)
                and i.engine == mybir.EngineType.SP
            ]
            assert entry_names, "SP barrier drain not found in entry block"
            entry.instructions.insert(entry_names[0][0], hoist)
            break

    # Tile's exit protocol waits on its DMASW0 lane semaphore for the SWDGE
    # writeback, but the descriptor carries a single completion sem —
    # on_update[0], which the sem= kwarg filled with our placeholder. Point
    # on_update[0] at the DMASW0 lane sem instead so the DMA completion
    # (data landed + sem propagation) is what releases the exit barrier, in
    # both the executor and the timeline cost model.
    fn = nc.m.functions[0]
    dmasw = None
    for b in fn.blocks:
        for inst in b.instructions:
            si = inst.sync_info
            if not si:
                continue
            for w in si.on_wait or []:
                nm = getattr(w, "ant_name", "") or ""
                if nm.startswith("DMASW"):
                    dmasw = w
    assert dmasw is not None, "no DMASW exit wait found"
    for b in fn.blocks:
        for inst in b.instructions:
            if isinstance(inst, mybir.InstKVWritebackAnt):
                inst.sync_info.on_update[0] = mybir.SyncUpdate(
                    sync_type="semaphore",
                    id=dmasw.id,
                    ant_name=dmasw.ant_name,
                    update_mode="sem-add-imm",
                    update_value=16,
                )

    nc.compile()
    return nc


def _pack31(bits):
    """bits [..., D] uint8 -> [..., WPK] float32-viewed words, 31 bits/word
    at positions 0..29 and 31 (bit 30 always zero -> never NaN/Inf)."""
    lead = bits.shape[:-1]
    b32 = np.zeros(lead + (WPK, 32), dtype=np.uint8)
    pad = np.zeros(lead + (DPAD,), dtype=np.uint8)
    pad[..., :D] = bits
    pad = pad.reshape(lead + (WPK, PAYLOAD))
    b32[..., :30] = pad[..., :30]
    b32[..., 31] = pad[..., 30]
    words = np.packbits(b32.reshape(lead + (WPK * 32,)), axis=-1, bitorder="little")
    return words.view(np.uint32).view(np.float32)


def _pack_inputs(x, bit_weights):
    x = np.asarray(x).astype(np.uint8)
    bw = np.ascontiguousarray(np.asarray(bit_weights).astype(np.uint8))
    notx = (1 - x).astype(np.uint8)
    nxp = _pack31(notx)  # [WPK]
    wp = _pack31(bw)  # [L, WPK]
    in_maps = []
    for i in range(NCORES):
        shard = wp[i * LSH : (i + 1) * LSH].reshape(128, NB, WPK)
        wx = np.empty((128, NB + 1, WPK), dtype=np.float32)
        wx[:, 0, :] = nxp
        wx[:, 1:, :] = shard
        in_maps.append({"wx": wx.reshape(128, (NB + 1) * WPK)})
    return in_maps


def _gather(results):
    outs = []
    for i in range(NCORES):
        # [1, 128, 1, NB] fp32 violation flags; res[0, p, 0, b] covers neuron
        # 8p + b, flag == 0.0 means no violated requirement -> output True
        res = results[i]["res"].reshape(128, NB)
        outs.append(res.reshape(-1) == 0.0)
    return np.concatenate(outs).astype(np.bool_)


def _get_compiled():
    global _compiled
    if _compiled is None:
        _compiled = _build()
    return _compiled


def kernel(x, bit_weights):
    from concourse import bass_utils

    nc = _get_compiled()
    in_maps = _pack_inputs(x, bit_weights)
    r = bass_utils.run_bass_kernel_spmd(nc, in_maps, core_ids=list(range(NCORES)))
    return _gather(r.results)

